# revision 14
# baseline (speedup 1.0000x reference)
"""MiniRocketFeatures Trainium2 Bass kernel (v2, L-pipelined).

Full inputs in, full outputs out; internally shards the batch (256) across
8 NeuronCores (32 batches per core), pure data parallel.

Per-core math (B=32 batches, C=23 channels, L=4096):
  s = x.sum(axis=1)                          # channel sum, via PE matmul
  for each of 12 (k_len, dilation) groups:
     conv = dilated window-sum of s (zero-padded, L_out == L)
     M[g] = conv.max(axis=-1)                # exact
     N[g] = min over a 64-position witness subset
  out[b, 2k]   = (M[g(k)] - bias[k] > 0)     # == reference f1
  out[b, 2k+1] = (M[g(k)] - N[g(k)] > 0)     # == reference f2 for any
                                             #    non-degenerate input
Final compare done as one (32,25)@(25,20480) matmul against a precomputed
selection matrix G followed by >0 thresholds (sigmoid sat / is_gt).

Layout: L is split in two halves (L-pipelining: conv of half A overlaps the
x DMA of half B). Per half, s lives in a 128-partition tile H: partition
p = 32*j + b (j = subchunk of 512), H col t <-> s position
Lbase + 512*j + (t - 128), with 128 halo columns on each side.

Window sums per dilation d use 5 DVE ops (all bf16, 2x mode):
  w2 = s + s(+d); w4 = w2 + w2(+2d); w8 = w4 + w4(+4d)   # taps 0..7
  c9[i] = w8[i+128-4d] + s[i+128+4d]                      # taps -4d..4d
  c7[i] = w8[i+128-3d] - s[i+128+4d]                      # taps -3d..3d
Group maxes via a TT-max tree over all 12 groups at once (cheaper than
DVE tensor_reduce which runs at 1x + drain).
"""

import os
import sys

import numpy as np


def _ensure_paths():
    for p in ("/opt/trn_rl_repo", "/root/.axon_site/_ro/trn_rl_repo"):
        if os.path.isdir(p) and p not in sys.path:
            sys.path.append(p)


_ensure_paths()

import ml_dtypes  # noqa: E402

import concourse.bacc as bacc  # noqa: E402
import concourse.mybir as mybir  # noqa: E402
import concourse.tile as tile  # noqa: E402

B_FULL, C, L = 256, 23, 4096
N_CORES = 8
B = B_FULL // N_CORES  # 32 batches per core
K_TOTAL = 10000
NF = 2 * K_TOTAL  # 20000 output features per batch
NFP = 20480  # NF padded for uniform chunking
DILS = (1, 2, 4, 8, 16, 32)
N_GROUPS = 12  # (k7, k9) x 6 dilations
SC = 512  # subchunk length
HW = 128 + SC + 128  # per-half H tile width (768)
HALF = 4 * SC  # L covered per half (2048)
N_CG = 6  # channel groups: 5 x 4ch + 1 x 3ch

F32 = mybir.dt.float32
F32R = mybir.dt.float32r
BF16 = mybir.dt.bfloat16

MCH = 512  # feature matmul free dim (one psum bank half)
MCH2 = 1024  # threshold chunk (per psum tile)
OCH = 4096  # out DMA chunk
# threshold engine per feature chunk: a=Act sigmoid, v=DVE is_gt
# (gpsimd cannot read PSUM -- BIR verifier rule)
THR_PATTERN = ("a", "a", "v", "a", "a", "a", "v", "a", "a", "v")
# dilations whose window-sum chain runs on gpsimd: EMPTY -- measured on HW,
# concurrent gpsimd streaming drops DVE tensor ops from 2x to 1x mode
# (SBUF bandwidth contention), a net loss.
GP_DILS = ()


def _config():
    """Deterministic stand-in for the np.random config drawn in __init__
    (mirrors the reference module exactly)."""
    rng = np.random.default_rng(0)
    kl = rng.choice(np.array([7, 9]), size=K_TOTAL)
    dil_exp = rng.integers(0, 6, size=K_TOTAL)
    dil = (2 ** dil_exp).astype(np.int64)
    biases = rng.uniform(-1.0, 1.0, size=K_TOTAL).astype(np.float32)
    return kl, dil, biases


def _build_consts():
    kl, dil, biases = _config()
    g_of = {}
    for di, d in enumerate(DILS):
        g_of[(7, d)] = 2 * di
        g_of[(9, d)] = 2 * di + 1
    # G rows: 0:12 coef of group max M_g, 12:24 coef of group min N_g,
    # 24 bias row (FT row 24 is constant 1.0).
    G = np.zeros((25, NFP), np.float32)
    ks = np.arange(K_TOTAL)
    gs = np.array([g_of[(int(k), int(d))] for k, d in zip(kl, dil)])
    G[gs, 2 * ks] = 1.0
    G[24, 2 * ks] = -biases
    # FT rows 12:24 carry the NEGATED witness min (-N_g), so both halves of
    # the spread M_g - N_g enter with coefficient +1.
    G[gs, 2 * ks + 1] = 1.0
    G[12 + gs, 2 * ks + 1] = 1.0

    # chansum lhsT: maps (b, c)-packed K partitions to output partition 32j+b
    # (j = subchunk index within the half); one [*, 512] array, col block j.
    wqa = np.zeros((128, 512), np.float32)  # 4-channel groups
    wqr = np.zeros((96, 512), np.float32)  # 3-channel remainder group
    for j in range(4):
        for b in range(32):
            wqa[b * 4 : b * 4 + 4, 128 * j + 32 * j + b] = 1.0
            wqr[b * 3 : b * 3 + 3, 128 * j + 32 * j + b] = 1.0
    eye = np.eye(128, dtype=np.float32)
    return (
        G.astype(ml_dtypes.bfloat16),
        wqa,
        wqr,
        eye.astype(ml_dtypes.bfloat16),
    )


def build_nc(debug=False):
    nc = bacc.Bacc("TRN2", target_bir_lowering=False, debug=debug)

    x_d = nc.dram_tensor("x", [B, C, L], F32R, kind="ExternalInput")
    g_d = nc.dram_tensor("g", [25, NFP], BF16, kind="ExternalInput")
    wa_d = nc.dram_tensor("wa", [128, 512], F32R, kind="ExternalInput")
    wr_d = nc.dram_tensor("wr", [96, 512], F32R, kind="ExternalInput")
    eye_d = nc.dram_tensor("eye", [128, 128], BF16, kind="ExternalInput")
    out_d = nc.dram_tensor("out", [B, NFP], BF16, kind="ExternalOutput")

    AL = mybir.AluOpType
    AX = mybir.AxisListType

    # per-half x tile column ranges (half A carries 128 extra cols: its own
    # right-halo source data, L 2048..2175)
    XC = (HALF + 128, HALF)
    XO = (0, HALF)

    with tile.TileContext(nc) as tc:
        with (
            tc.tile_pool(name="persist", bufs=1) as pp,
            tc.tile_pool(name="xt", bufs=1) as xp,
            tc.tile_pool(name="conv", bufs=2) as cp,
            tc.tile_pool(name="tree", bufs=1) as tp,
            tc.tile_pool(name="fin", bufs=3) as fp,
            tc.tile_pool(name="pcs", bufs=1, space="PSUM") as pcs,
            tc.tile_pool(name="ptr", bufs=1, space="PSUM") as ptr,
            tc.tile_pool(name="psv", bufs=2, space="PSUM") as psv,
        ):
            # ---- persistent tiles ----
            H = [pp.tile([128, HW], BF16, tag=f"H{s}", name=f"H{s}") for s in (0, 1)]
            cv = [
                pp.tile([128, N_GROUPS, SC], BF16, tag=f"cv{s}", name=f"cv{s}")
                for s in (0, 1)
            ]
            rmc = pp.tile([128, 24], BF16, tag="rmc")
            ra = pp.tile([128, N_GROUPS], BF16, tag="ra")
            rb = pp.tile([128, N_GROUPS], BF16, tag="rb")
            sb_t = pp.tile([24, 4, 32], BF16, tag="sbt")
            FT = pp.tile([25, 32], BF16, tag="FT")
            wa_t = pp.tile([128, 512], F32R, tag="wa")
            wr_t = pp.tile([96, 512], F32R, tag="wr")
            g_t = pp.tile([25, NFP], BF16, tag="G")
            eye_t = pp.tile([128, 128], BF16, tag="eye")

            # ---- t=0: edge memsets (global zero padding) ----
            nc.vector.memset(H[0][0:32, 0:128], 0.0)
            nc.vector.memset(H[1][96:128, 640:768], 0.0)

            # ---- weights first on the Act HW queue ----
            nc.scalar.dma_start(wa_t[:], wa_d[:])
            nc.scalar.dma_start(wr_t[:], wr_d[:])

            # ---- x DMAs split across BOTH HWDGE queues (SP + Act): one
            # queue's entry-generation cannot keep 16 DMA engines fed
            # (measured ~230 GB/s with inter-entry gaps; two queues overlap
            # generation with transfer) ----
            xts = {}
            for s in (0, 1):
                for cg in range(N_CG):
                    ncch = 4 if cg < 5 else 3
                    t = xp.tile(
                        [32 * ncch, XC[s]],
                        F32R,
                        tag=f"x{cg}",
                        name=f"x{s}_{cg}",
                    )
                    eng = nc.sync if cg % 2 == 0 else nc.scalar
                    eng.dma_start(
                        t[:, :],
                        x_d[:, 4 * cg : 4 * cg + ncch, XO[s] : XO[s] + XC[s]],
                    )
                    xts[(s, cg)] = t

            # G / eye ride behind the x stream (not needed until the tail)
            nc.scalar.dma_start(g_t[:], g_d[:])
            nc.scalar.dma_start(eye_t[:], eye_d[:])

            def w_of(cg):
                return wa_t if cg < 5 else wr_t

            def chansum(s):
                pm = pcs.tile([128, SC], F32, tag="pm", name=f"pm{s}")
                n_mm = 4 * N_CG
                i = 0
                for cg in range(N_CG):
                    for j in range(4):
                        nc.tensor.matmul(
                            pm[:, :],
                            w_of(cg)[:, 128 * j : 128 * j + 128],
                            xts[(s, cg)][:, SC * j : SC * j + SC],
                            start=(i == 0),
                            stop=(i == n_mm - 1),
                        )
                        i += 1
                return pm

            # ---- half A: chansum + right-edge, copies, halos ----
            pmA = chansum(0)
            peA = pcs.tile([128, 128], F32, tag="pe")
            for cg in range(N_CG):
                nc.tensor.matmul(
                    peA[:, :],
                    w_of(cg)[:, 384:512],
                    xts[(0, cg)][:, HALF : HALF + 128],
                    start=(cg == 0),
                    stop=(cg == N_CG - 1),
                )
            nc.scalar.copy(H[0][:, 128:640], pmA[:, :])
            nc.scalar.copy(H[0][96:128, 640:768], peA[96:128, :])
            # halo DMAs ride the gpsimd SWDGE queue (empty; the SP queue is
            # busy streaming x and is strictly FIFO)
            nc.gpsimd.dma_start(H[0][32:128, 0:128], H[0][0:96, 512:640])
            nc.gpsimd.dma_start(H[0][0:96, 640:768], H[0][32:128, 128:256])

            # ---- conv + max tree ----
            def conv_half(s):
                Hs = H[s]
                for di, d in enumerate(DILS):
                    e = nc.gpsimd if d in GP_DILS else nc.vector
                    g7, g9 = 2 * di, 2 * di + 1
                    w2 = cp.tile([128, HW], BF16, tag=f"w2{d in GP_DILS}", name=f"w2_{s}_{d}")
                    w4 = cp.tile([128, HW], BF16, tag=f"w4{d in GP_DILS}", name=f"w4_{s}_{d}")
                    w8 = cp.tile([128, HW], BF16, tag=f"w8{d in GP_DILS}", name=f"w8_{s}_{d}")
                    W2, W4, W8 = HW - d, HW - 3 * d, HW - 7 * d
                    e.tensor_add(w2[:, 0:W2], Hs[:, 0:W2], Hs[:, d : d + W2])
                    e.tensor_add(
                        w4[:, 0:W4], w2[:, 0:W4], w2[:, 2 * d : 2 * d + W4]
                    )
                    e.tensor_add(
                        w8[:, 0:W8], w4[:, 0:W8], w4[:, 4 * d : 4 * d + W8]
                    )
                    ht = Hs[:, 128 + 4 * d : 640 + 4 * d]
                    e.tensor_add(
                        cv[s][:, g9, :], w8[:, 128 - 4 * d : 640 - 4 * d], ht
                    )
                    e.tensor_tensor(
                        cv[s][:, g7, :],
                        w8[:, 128 - 3 * d : 640 - 3 * d],
                        ht,
                        op=AL.subtract,
                    )

            def max_tree(s, out):
                cur = cv[s][:, :, :]
                width = SC
                lvl = 0
                while width > 8:
                    width //= 2
                    nxt = tp.tile(
                        [128, N_GROUPS, width], BF16, tag=f"t{lvl}", name=f"t{lvl}_{s}"
                    )
                    nc.vector.tensor_max(
                        nxt[:, :, :], cur[:, :, 0:width], cur[:, :, width : 2 * width]
                    )
                    cur = nxt
                    lvl += 1
                nc.vector.tensor_reduce(out[:], cur[:, :, :], axis=AX.X, op=AL.max)

            conv_half(0)
            max_tree(0, ra)
            # spread witness: -min over the first 16 conv values of each
            # subchunk, stored negated so the cross-subchunk combine is a
            # single max-reduce from partition 0 (HW: engine SBUF ops may
            # only start at partition 0/32/64/96).
            negc = pp.tile([128, N_GROUPS, 16], BF16, tag="negc")
            nc.vector.tensor_scalar(
                negc[:, :, :], cv[0][:, :, 0:16], -1.0, None, op0=AL.mult
            )
            nc.vector.tensor_reduce(
                rmc[:, 12:24], negc[:, :, :], axis=AX.X, op=AL.max
            )

            # ---- half B ----
            pmB = chansum(1)
            nc.scalar.copy(H[1][:, 128:640], pmB[:, :])
            nc.gpsimd.dma_start(H[1][0:32, 0:128], H[0][96:128, 512:640])
            nc.gpsimd.dma_start(H[1][32:128, 0:128], H[1][0:96, 512:640])
            nc.gpsimd.dma_start(H[1][0:96, 640:768], H[1][32:128, 128:256])

            conv_half(1)
            max_tree(1, rb)
            nc.vector.tensor_max(rmc[:, 0:12], ra[:], rb[:])

            # ---- combine across subchunks via PE transpose ----
            pt = ptr.tile([24, 128], BF16, tag="pt")
            nc.tensor.transpose(pt[:], rmc[:], eye_t[:])
            nc.scalar.copy(sb_t[:, :, :], pt[:])
            # rows 0:12 combine maxes (max over j); rows 12:24 combine the
            # negated mins (max over j == -min). One reduce from partition 0.
            nc.vector.memset(FT[:, :], 1.0)
            nc.vector.tensor_reduce(
                FT[0:24, :],
                sb_t[0:24, :, :].rearrange("p j b -> p b j"),
                axis=AX.X,
                op=AL.max,
            )

            # ---- feature matmul + threshold + out ----
            # psum chunks of 1024 (2 matmuls each, both within one bank
            # half); thresholds drain on Act (sigmoid sat) + DVE (is_gt).
            osb = {}
            for mc in range(NFP // MCH2):
                vps = psv.tile([32, MCH2], F32, tag="fv", name=f"fv{mc}")
                for s2 in range(MCH2 // MCH):
                    nc.tensor.matmul(
                        vps[:, MCH * s2 : MCH * (s2 + 1)],
                        FT[0:25, :],
                        g_t[:, MCH2 * mc + MCH * s2 : MCH2 * mc + MCH * (s2 + 1)],
                        start=True,
                        stop=True,
                    )
                oc = mc // (OCH // MCH2)
                if mc % (OCH // MCH2) == 0:
                    osb[oc] = fp.tile([32, OCH], BF16, tag="osb", name=f"osb{oc}")
                dst = osb[oc][
                    :, MCH2 * (mc % (OCH // MCH2)) : MCH2 * (mc % (OCH // MCH2)) + MCH2
                ]
                eng = THR_PATTERN[mc % len(THR_PATTERN)]
                if eng == "a":
                    nc.scalar.activation(
                        dst, vps[:], mybir.ActivationFunctionType.Sigmoid, scale=1000.0
                    )
                else:
                    nc.vector.tensor_scalar(dst, vps[:], 0.0, None, op0=AL.is_gt)
                if mc % (OCH // MCH2) == (OCH // MCH2) - 1:
                    nc.sync.dma_start(
                        out_d[:, OCH * oc : OCH * (oc + 1)], osb[oc][:]
                    )
    nc.compile()
    return nc


_CACHE = {}


def _get_nc():
    if "nc" not in _CACHE:
        _CACHE["nc"] = build_nc(debug=False)
        _CACHE["consts"] = _build_consts()
    return _CACHE["nc"], _CACHE["consts"]


def _run(x, trace=False, tmpdir=None):
    from concourse.bass_utils import run_bass_kernel_spmd

    nc, (G, wa, wr, eye) = _get_nc()
    x = np.ascontiguousarray(np.asarray(x), dtype=np.float32)
    assert x.shape == (B_FULL, C, L), x.shape
    in_maps = [
        {
            "x": np.ascontiguousarray(x[B * i : B * (i + 1)]),
            "g": G,
            "wa": wa,
            "wr": wr,
            "eye": eye,
        }
        for i in range(N_CORES)
    ]
    res = run_bass_kernel_spmd(
        nc, in_maps, core_ids=list(range(N_CORES)), trace=trace, tmpdir=tmpdir
    )
    out = np.empty((B_FULL, NF, 1), np.float32)
    for i in range(N_CORES):
        out[B * i : B * (i + 1), :, 0] = res.results[i]["out"][:, :NF].astype(np.float32)
    return out, res


def kernel(x):
    out, _ = _run(x, trace=False)
    return out


# revision 23
# speedup vs baseline: 1.1163x; 1.1163x over previous
"""MiniRocketFeatures Trainium2 Bass kernel (v2, L-pipelined).

Full inputs in, full outputs out; internally shards the batch (256) across
8 NeuronCores (32 batches per core), pure data parallel.

Per-core math (B=32 batches, C=23 channels, L=4096):
  s = x.sum(axis=1)                          # channel sum, via PE matmul
  for each of 12 (k_len, dilation) groups:
     conv = dilated window-sum of s (zero-padded, L_out == L)
     M[g] = conv.max(axis=-1)                # exact
     N[g] = min over a 64-position witness subset
  out[b, 2k]   = (M[g(k)] - bias[k] > 0)     # == reference f1
  out[b, 2k+1] = (M[g(k)] - N[g(k)] > 0)     # == reference f2 for any
                                             #    non-degenerate input
Final compare done as one (32,25)@(25,20480) matmul against a precomputed
selection matrix G followed by >0 thresholds (sigmoid sat / is_gt).

Layout: L is split in two halves (L-pipelining: conv of half A overlaps the
x DMA of half B). Per half, s lives in a 128-partition tile H: partition
p = 32*j + b (j = subchunk of 512), H col t <-> s position
Lbase + 512*j + (t - 128), with 128 halo columns on each side.

Window sums per dilation d use 5 DVE ops (all bf16, 2x mode):
  w2 = s + s(+d); w4 = w2 + w2(+2d); w8 = w4 + w4(+4d)   # taps 0..7
  c9[i] = w8[i+128-4d] + s[i+128+4d]                      # taps -4d..4d
  c7[i] = w8[i+128-3d] - s[i+128+4d]                      # taps -3d..3d
Group maxes via a TT-max tree over all 12 groups at once (cheaper than
DVE tensor_reduce which runs at 1x + drain).
"""

import os
import sys

import numpy as np


def _ensure_paths():
    for p in ("/opt/trn_rl_repo", "/root/.axon_site/_ro/trn_rl_repo"):
        if os.path.isdir(p) and p not in sys.path:
            sys.path.append(p)


_ensure_paths()

import ml_dtypes  # noqa: E402

import concourse.bacc as bacc  # noqa: E402
import concourse.mybir as mybir  # noqa: E402
import concourse.tile as tile  # noqa: E402

B_FULL, C, L = 256, 23, 4096
N_CORES = 8
B = B_FULL // N_CORES  # 32 batches per core
K_TOTAL = 10000
NF = 2 * K_TOTAL  # 20000 output features per batch
NFP = 20480  # NF padded for uniform chunking
DILS = (1, 2, 4, 8, 16, 32)
N_GROUPS = 12  # (k7, k9) x 6 dilations
SC = 512  # subchunk length
HW = 128 + SC + 128  # per-half H tile width (768)
HALF = 4 * SC  # L covered per half (2048)
N_CG = 6  # channel groups: 5 x 4ch + 1 x 3ch

F32 = mybir.dt.float32
F32R = mybir.dt.float32r
BF16 = mybir.dt.bfloat16

MCH = 512  # feature matmul free dim (one psum bank half)
MCH2 = 1024  # threshold chunk (per psum tile)
OCH = 4096  # out DMA chunk
# threshold engine per feature chunk: a=Act sigmoid, v=DVE is_gt
# (gpsimd cannot read PSUM -- BIR verifier rule)
THR_PATTERN = ("a", "a", "v", "a", "a", "a", "v", "a", "a", "v")
USE_TTR = False
# NOTE: all conv elementwise work stays on DVE -- measured on HW, concurrent
# gpsimd streaming drops DVE tensor ops from 2x to 1x mode (SBUF bandwidth
# contention), a net loss.


def _config():
    """Deterministic stand-in for the np.random config drawn in __init__
    (mirrors the reference module exactly)."""
    rng = np.random.default_rng(0)
    kl = rng.choice(np.array([7, 9]), size=K_TOTAL)
    dil_exp = rng.integers(0, 6, size=K_TOTAL)
    dil = (2 ** dil_exp).astype(np.int64)
    biases = rng.uniform(-1.0, 1.0, size=K_TOTAL).astype(np.float32)
    return kl, dil, biases


def _build_consts():
    kl, dil, biases = _config()
    g_of = {}
    for di, d in enumerate(DILS):
        g_of[(7, d)] = 2 * di
        g_of[(9, d)] = 2 * di + 1
    # G rows: 0:12 coef of group max M_g, 12:24 coef of group min N_g,
    # 24 bias row (FT row 24 is constant 1.0).
    G = np.zeros((25, NFP), np.float32)
    ks = np.arange(K_TOTAL)
    gs = np.array([g_of[(int(k), int(d))] for k, d in zip(kl, dil)])
    G[gs, 2 * ks] = 1.0
    G[24, 2 * ks] = -biases
    # FT rows 12:24 carry the NEGATED witness min (-N_g), so both halves of
    # the spread M_g - N_g enter with coefficient +1.
    G[gs, 2 * ks + 1] = 1.0
    G[12 + gs, 2 * ks + 1] = 1.0

    # chansum lhsT: maps (b, c)-packed K partitions to output partition 32j+b
    # (j = subchunk index within the half); one [*, 512] array, col block j.
    wqa = np.zeros((128, 512), np.float32)  # 4-channel groups
    wqr = np.zeros((96, 512), np.float32)  # 3-channel remainder group
    for j in range(4):
        for b in range(32):
            wqa[b * 4 : b * 4 + 4, 128 * j + 32 * j + b] = 1.0
            wqr[b * 3 : b * 3 + 3, 128 * j + 32 * j + b] = 1.0
    eye = np.eye(128, dtype=np.float32)
    return (
        G.astype(ml_dtypes.bfloat16),
        wqa,
        wqr,
        eye.astype(ml_dtypes.bfloat16),
    )


def build_nc(debug=False):
    nc = bacc.Bacc("TRN2", target_bir_lowering=False, debug=debug)

    x_d = nc.dram_tensor("x", [B, C, L], F32R, kind="ExternalInput")
    g_d = nc.dram_tensor("g", [25, NFP], BF16, kind="ExternalInput")
    wa_d = nc.dram_tensor("wa", [128, 512], F32R, kind="ExternalInput")
    wr_d = nc.dram_tensor("wr", [96, 512], F32R, kind="ExternalInput")
    eye_d = nc.dram_tensor("eye", [128, 128], BF16, kind="ExternalInput")
    out_d = nc.dram_tensor("out", [B, NFP], BF16, kind="ExternalOutput")

    AL = mybir.AluOpType
    AX = mybir.AxisListType

    # per-half x tile column ranges (half A carries 128 extra cols: its own
    # right-halo source data, L 2048..2175)
    XC = (HALF + 128, HALF)
    XO = (0, HALF)

    with tile.TileContext(nc) as tc:
        with (
            tc.tile_pool(name="persist", bufs=1) as pp,
            tc.tile_pool(name="xt", bufs=1) as xp,
            tc.tile_pool(name="conv", bufs=2) as cp,
            tc.tile_pool(name="tree", bufs=1) as tp,
            tc.tile_pool(name="fin", bufs=3) as fp,
            tc.tile_pool(name="pcs", bufs=1, space="PSUM") as pcs,
            tc.tile_pool(name="ptr", bufs=1, space="PSUM") as ptr,
            tc.tile_pool(name="psv", bufs=2, space="PSUM") as psv,
        ):
            # ---- persistent tiles ----
            H = [pp.tile([128, HW], BF16, tag=f"H{s}", name=f"H{s}") for s in (0, 1)]
            cv = [
                pp.tile([128, N_GROUPS, SC], BF16, tag=f"cv{s}", name=f"cv{s}")
                for s in (0, 1)
            ]
            rmc = pp.tile([128, 24], BF16, tag="rmc")
            ra = pp.tile([128, N_GROUPS], BF16, tag="ra")
            rb = pp.tile([128, N_GROUPS], BF16, tag="rb")
            sb_t = pp.tile([24, 4, 32], BF16, tag="sbt")
            FT = pp.tile([25, 32], BF16, tag="FT")
            wa_t = pp.tile([128, 512], F32R, tag="wa")
            wr_t = pp.tile([96, 512], F32R, tag="wr")
            g_t = pp.tile([25, NFP], BF16, tag="G")
            eye_t = pp.tile([128, 128], BF16, tag="eye")

            # ---- t=0: edge memsets (global zero padding) ----
            nc.vector.memset(H[0][0:32, 0:128], 0.0)
            nc.vector.memset(H[1][96:128, 640:768], 0.0)

            # ---- weights first on the Act HW queue ----
            nc.scalar.dma_start(wa_t[:], wa_d[:])
            nc.scalar.dma_start(wr_t[:], wr_d[:])

            # ---- x DMAs split across BOTH HWDGE queues (SP + Act): one
            # queue's entry-generation cannot keep 16 DMA engines fed
            # (measured ~230 GB/s with inter-entry gaps; two queues overlap
            # generation with transfer). Half A first on both queues. ----
            xts = {}

            def x_dmas(s):
                for cg in range(N_CG):
                    ncch = 4 if cg < 5 else 3
                    t = xp.tile(
                        [32 * ncch, XC[s]],
                        F32R,
                        tag=f"x{cg}",
                        name=f"x{s}_{cg}",
                    )
                    eng = nc.sync if cg % 2 == 0 else nc.scalar
                    eng.dma_start(
                        t[:, :],
                        x_d[:, 4 * cg : 4 * cg + ncch, XO[s] : XO[s] + XC[s]],
                    )
                    xts[(s, cg)] = t

            x_dmas(0)

            def w_of(cg):
                return wa_t if cg < 5 else wr_t

            def chansum(s, pe=None):
                pm = pcs.tile([128, SC], F32, tag="pm", name=f"pm{s}")
                n_mm = 4 * N_CG
                i = 0
                for cg in range(N_CG):
                    for j in range(4):
                        nc.tensor.matmul(
                            pm[:, :],
                            w_of(cg)[:, 128 * j : 128 * j + 128],
                            xts[(s, cg)][:, SC * j : SC * j + SC],
                            start=(i == 0),
                            stop=(i == n_mm - 1),
                        )
                        i += 1
                if pe is not None:
                    for cg in range(N_CG):
                        nc.tensor.matmul(
                            pe[:, :],
                            w_of(cg)[:, 384:512],
                            xts[(s, cg)][:, HALF : HALF + 128],
                            start=(cg == 0),
                            stop=(cg == N_CG - 1),
                        )
                return pm

            # ---- half A: chansum + right-edge, copies, halos ----
            peA = pcs.tile([128, 128], F32, tag="pe")
            pmA = chansum(0, pe=peA)
            nc.scalar.copy(H[0][:, 128:640], pmA[:, :])
            nc.scalar.copy(H[0][96:128, 640:768], peA[96:128, :])
            # halo DMAs ride the SP HWDGE queue BETWEEN the half-A and
            # half-B x entries: the gpsimd SWDGE queue trickles packets
            # (~6 us for these small copies); HWDGE does them in ~0.3 us.
            # The Act queue keeps streaming x while SP waits here.
            nc.sync.dma_start(H[0][32:128, 0:128], H[0][0:96, 512:640])
            nc.sync.dma_start(H[0][0:96, 640:768], H[0][32:128, 128:256])
            nc.sync.dma_start(H[1][0:32, 0:128], H[0][96:128, 512:640])

            # ---- half B x stream + consts ----
            x_dmas(1)
            nc.scalar.dma_start(g_t[:], g_d[:])
            nc.scalar.dma_start(eye_t[:], eye_d[:])

            # ---- conv: per-dilation minimal windows + TTR-fused maxes ----
            # Ops run over H cols [128-4d, 640+4d) only (the exact reach of
            # the two conv taps). In w-tile coords (base a0 = 128-4d):
            #   w2[t] = H[a0+t] + H[a0+t+d]        t in [0, 512+7d)
            #   w4[t] = w2[t] + w2[t+2d]           t in [0, 512+5d)
            #   w8[t] = w4[t] + w4[t+4d]           t in [0, 512+d)
            #   c9[i] = w8[i] + H[128+4d+i]        (TTR, accum max -> acc[g9])
            #   c7[i] = w8[i+d] - H[128+4d+i]      (TTR, accum max -> acc[g7])
            NEG = -1.0e30

            def conv_half(s, acc):
                Hs = H[s]
                for di, d in enumerate(DILS):
                    e = nc.vector
                    g7, g9 = 2 * di, 2 * di + 1
                    a0 = 128 - 4 * d
                    w2 = cp.tile([128, HW], BF16, tag="w2", name=f"w2_{s}_{d}")
                    w4 = cp.tile([128, HW], BF16, tag="w4", name=f"w4_{s}_{d}")
                    w8 = cp.tile([128, HW], BF16, tag="w8", name=f"w8_{s}_{d}")
                    W2, W4, W8 = SC + 7 * d, SC + 5 * d, SC + d
                    e.tensor_add(
                        w2[:, 0:W2], Hs[:, a0 : a0 + W2], Hs[:, a0 + d : a0 + d + W2]
                    )
                    e.tensor_add(
                        w4[:, 0:W4], w2[:, 0:W4], w2[:, 2 * d : 2 * d + W4]
                    )
                    e.tensor_add(
                        w8[:, 0:W8], w4[:, 0:W8], w4[:, 4 * d : 4 * d + W8]
                    )
                    ht = Hs[:, 128 + 4 * d : 640 + 4 * d]
                    if USE_TTR:
                        e.tensor_tensor_reduce(
                            cv[s][:, g9, :], w8[:, 0:SC], ht,
                            1.0, NEG, AL.add, AL.max, acc[:, g9 : g9 + 1],
                        )
                        e.tensor_tensor_reduce(
                            cv[s][:, g7, :], w8[:, d : d + SC], ht,
                            1.0, NEG, AL.subtract, AL.max, acc[:, g7 : g7 + 1],
                        )
                    else:
                        e.tensor_add(cv[s][:, g9, :], w8[:, 0:SC], ht)
                        e.tensor_tensor(
                            cv[s][:, g7, :], w8[:, d : d + SC], ht, op=AL.subtract
                        )

            def max_tree(s, out):
                cur = cv[s][:, :, :]
                width = SC
                lvl = 0
                while width > 8:
                    width //= 2
                    nxt = tp.tile(
                        [128, N_GROUPS, width], BF16, tag=f"t{lvl}", name=f"t{lvl}_{s}"
                    )
                    nc.vector.tensor_max(
                        nxt[:, :, :], cur[:, :, 0:width], cur[:, :, width : 2 * width]
                    )
                    cur = nxt
                    lvl += 1
                nc.vector.tensor_reduce(out[:], cur[:, :, :], axis=AX.X, op=AL.max)

            conv_half(0, ra)
            if not USE_TTR:
                max_tree(0, ra)
            # spread witness: -min over the first 16 conv values of each
            # subchunk, stored negated so the cross-subchunk combine is a
            # single max-reduce from partition 0 (HW: engine SBUF ops may
            # only start at partition 0/32/64/96).
            negc = pp.tile([128, N_GROUPS, 16], BF16, tag="negc")
            nc.vector.tensor_scalar(
                negc[:, :, :], cv[0][:, :, 0:16], -1.0, None, op0=AL.mult
            )
            nc.vector.tensor_reduce(
                rmc[:, 12:24], negc[:, :, :], axis=AX.X, op=AL.max
            )

            # ---- half B ----
            pmB = chansum(1)
            nc.scalar.copy(H[1][:, 128:640], pmB[:, :])
            nc.sync.dma_start(H[1][32:128, 0:128], H[1][0:96, 512:640])
            nc.sync.dma_start(H[1][0:96, 640:768], H[1][32:128, 128:256])

            conv_half(1, rb)
            if not USE_TTR:
                max_tree(1, rb)
            nc.vector.tensor_max(rmc[:, 0:12], ra[:], rb[:])

            # ---- combine across subchunks via PE transpose ----
            pt = ptr.tile([24, 128], BF16, tag="pt")
            nc.tensor.transpose(pt[:], rmc[:], eye_t[:])
            nc.scalar.copy(sb_t[:, :, :], pt[:])
            # rows 0:12 combine maxes (max over j); rows 12:24 combine the
            # negated mins (max over j == -min). One reduce from partition 0.
            nc.vector.memset(FT[:, :], 1.0)
            nc.vector.tensor_reduce(
                FT[0:24, :],
                sb_t[0:24, :, :].rearrange("p j b -> p b j"),
                axis=AX.X,
                op=AL.max,
            )

            # ---- feature matmul + threshold + out ----
            # psum chunks of 1024 (2 matmuls each, both within one bank
            # half); thresholds drain on Act (sigmoid sat) + DVE (is_gt).
            osb = {}
            for mc in range(NFP // MCH2):
                vps = psv.tile([32, MCH2], F32, tag="fv", name=f"fv{mc}")
                for s2 in range(MCH2 // MCH):
                    nc.tensor.matmul(
                        vps[:, MCH * s2 : MCH * (s2 + 1)],
                        FT[0:25, :],
                        g_t[:, MCH2 * mc + MCH * s2 : MCH2 * mc + MCH * (s2 + 1)],
                        start=True,
                        stop=True,
                    )
                oc = mc // (OCH // MCH2)
                if mc % (OCH // MCH2) == 0:
                    osb[oc] = fp.tile([32, OCH], BF16, tag="osb", name=f"osb{oc}")
                dst = osb[oc][
                    :, MCH2 * (mc % (OCH // MCH2)) : MCH2 * (mc % (OCH // MCH2)) + MCH2
                ]
                eng = THR_PATTERN[mc % len(THR_PATTERN)]
                if eng == "a":
                    nc.scalar.activation(
                        dst, vps[:], mybir.ActivationFunctionType.Sigmoid, scale=1000.0
                    )
                else:
                    nc.vector.tensor_scalar(dst, vps[:], 0.0, None, op0=AL.is_gt)
                if mc % (OCH // MCH2) == (OCH // MCH2) - 1:
                    nc.sync.dma_start(
                        out_d[:, OCH * oc : OCH * (oc + 1)], osb[oc][:]
                    )
    nc.compile()
    return nc


_CACHE = {}


def _get_nc():
    if "nc" not in _CACHE:
        _CACHE["nc"] = build_nc(debug=False)
        _CACHE["consts"] = _build_consts()
    return _CACHE["nc"], _CACHE["consts"]


def _run(x, trace=False, tmpdir=None):
    from concourse.bass_utils import run_bass_kernel_spmd

    nc, (G, wa, wr, eye) = _get_nc()
    x = np.ascontiguousarray(np.asarray(x), dtype=np.float32)
    assert x.shape == (B_FULL, C, L), x.shape
    in_maps = [
        {
            "x": np.ascontiguousarray(x[B * i : B * (i + 1)]),
            "g": G,
            "wa": wa,
            "wr": wr,
            "eye": eye,
        }
        for i in range(N_CORES)
    ]
    res = run_bass_kernel_spmd(
        nc, in_maps, core_ids=list(range(N_CORES)), trace=trace, tmpdir=tmpdir
    )
    out = np.empty((B_FULL, NF, 1), np.float32)
    for i in range(N_CORES):
        out[B * i : B * (i + 1), :, 0] = res.results[i]["out"][:, :NF].astype(np.float32)
    return out, res


def kernel(x):
    out, _ = _run(x, trace=False)
    return out
